# revision 39
# baseline (speedup 1.0000x reference)
"""Trainium2 Bass kernel for nn_HermiteConvolution (dense_cnn).

Data-parallel over batch: 8 NeuronCores x 4 samples each. BatchNorm batch
statistics are made exact with two tiny (<=2KB) cross-core AllReduces.

SBUF layouts (all 128-partition packed; per-partition free bytes are the
scarce resource):
  x1q [128, 2056]  rows (q, b, o): x1 with L split in 4 quarters of 2048,
                   4-col halos on both sides of each quarter.
  X3  [96, 4, 2056] rows (dj, b, c): X3[dj,q,v] = x1q[(q,b,c), v+dj]
  X3e [96, 4, 2056] rows (dj2,b,c): X3e[dj2,q,v] = x1q[(q,b,c), v+2*dj2]
  a_pre/b_pre/c_pre [128, 4096] rows (h, br, b, o): L split in 2 halves.

Per 512-chunk k: q=k//4, h=k//8. Matmul outputs land at partition offsets
q*32 / h*64 (legal PE tile positions), so every compute op is lane-aligned;
only GPSIMD does partition-moving copies (stack builds, halos).

Phases: GEMM+branch convs+bn_stats (overlapped with x-stream) -> AllReduce#1
-> relu+px-conv+bn_stats -> AllReduce#2 -> relu+pool (ACT accum_out), fc,
Hermite kernel [4,9], scatter into 3 block-diag lhsT [96,4], final K=9 conv
as 3 accumulating matmuls per chunk over X3.

Returns (out [32,1,8192], x1 [32,8,8192]) matching the reference tuple.
"""

import math
import numpy as np
from contextlib import ExitStack

import concourse.bass as bass
import concourse.bacc as bacc
import concourse.tile as tile
import concourse.mybir as mybir
from concourse.bass_utils import run_bass_kernel_spmd

f32 = mybir.dt.float32
AF = mybir.ActivationFunctionType
ALU = mybir.AluOpType

NCORES = 8
B = 32
BS = B // NCORES          # 4 samples per core
CIN = 256
CO = 8                    # ORDER / channels
L = 8192
K = 9
LC = 512                  # matmul chunk
NCH = L // LC             # 16
NIC = CIN // 32           # 8 i-chunks of 32
LQ = L // 4               # 2048 per quarter
LH = L // 2               # 4096 per half
QF = LQ + 8               # 2056 quarter free size (4-col halos)
EPS = 1e-5
NTOT = float(B * L)

_CACHE = {}


def _build_nc():
    nc = bacc.Bacc("TRN2", target_bir_lowering=False, debug=False,
                   num_devices=NCORES)

    ap = {}
    def din(name, shape):
        ap[name] = nc.dram_tensor(name, shape, f32, kind="ExternalInput").ap()
    def dout(name, shape):
        ap[name] = nc.dram_tensor(name, shape, f32, kind="ExternalOutput").ap()

    # all small constants ride in one packed tensor -> one DMA -> one
    # semaphore lane (the LDWEIGHTS ISA slot fits a single sync wait)
    cshapes = {
        "c1b128": (128, 1), "lhsT_c1": (96, 64), "lhsT_c2": (96, 64),
        "lhsT_selB": (128, 128), "lhsT_px": (128, 64), "lhsT_fcw": (128, 4),
        "lhsT_fcc": (128, 4), "bvec_c1": (128, 1), "bvec_c2": (128, 1),
        "bvec_px": (128, 1), "gb_a": (128, 2), "gb_b": (128, 2),
        "gb_c": (128, 2), "fcb_w": (4, 1), "fcb_c": (4, 1), "c2b4": (4, 1),
        "ident4": (4, 4), "sel_dj": (72, 288), "mask96": (96, 4),
    }
    coffs = {}
    off = 0
    for name, (p, fdim) in cshapes.items():
        coffs[name] = off
        off += fdim
    CPACK_F = off

    din("xp", [128, NCH, NIC * LC])    # [(b,i32), chunk, (ic, s)] host-prepped
    din("wbd", [NIC, 128, 32])
    din("cpack", [128, CPACK_F])
    dout("out", [BS, L])
    dout("x1_out", [32, L])

    with tile.TileContext(nc) as tc, ExitStack() as ctx:
        const = ctx.enter_context(tc.tile_pool(name="const", bufs=1))
        persist = ctx.enter_context(tc.tile_pool(name="persist", bufs=1))
        small = ctx.enter_context(tc.tile_pool(name="small", bufs=1))
        dram = ctx.enter_context(tc.tile_pool(name="dram", bufs=1, space="DRAM"))
        pssm = ctx.enter_context(tc.tile_pool(name="pssm", bufs=1, space="PSUM"))

        # ---- constant loads (two DMAs total) ----
        w_sb = const.tile([128, NIC, 32], f32)
        nc.sync.dma_start(out=w_sb, in_=ap["wbd"].rearrange("ic p m -> p ic m"))
        cpack_sb = const.tile([128, CPACK_F], f32)
        nc.sync.dma_start(out=cpack_sb, in_=ap["cpack"])
        csb = {name: cpack_sb[0:p, coffs[name]:coffs[name] + fdim]
               for name, (p, fdim) in cshapes.items()}
        eps128 = const.tile([128, 1], f32)
        nc.vector.memset(eps128, EPS)

        # ---- persistent buffers ----
        x1q = persist.tile([128, QF], f32)
        X3 = persist.tile([96, 4, QF], f32)
        X3e = persist.tile([96, 4, QF], f32)
        a_pre = persist.tile([128, LH], f32)
        b_pre = persist.tile([128, LH], f32)
        c_pre = persist.tile([128, LH], f32)
        st_a = persist.tile([128, NCH // 2, 6], f32)
        st_b = persist.tile([128, NCH // 2, 6], f32)
        st_c = persist.tile([128, NCH // 2, 6], f32)

        nc.vector.memset(x1q[0:32, 0:4], 0.0)          # left pad of quarter 0
        nc.vector.memset(x1q[96:128, LQ + 4:QF], 0.0)  # right pad of quarter 3

        # ================= phase 1 =================
        # Stage A(k): x-DMA, conv1 GEMM, x1 chunk, stack copies (+ halo /
        # prefix / suffix fixups at quarter boundaries). Stage B(k): branch
        # convs + stats. B lags A by one chunk: the convs' right taps read
        # the first x1 columns of the NEXT chunk, and Tile preserves
        # program-order semantics, so B(k) must be emitted after A(k+1).
        with ExitStack() as p1:
            xpool = p1.enter_context(tc.tile_pool(name="xpool", bufs=3))
            ps1 = p1.enter_context(tc.tile_pool(name="ps1", bufs=2, space="PSUM"))

            # warm PE's view of the two const DMA lanes so later matmuls
            # (whose LDWEIGHTS slot fits only ONE sync wait) never need a
            # const wait on top of their data wait
            ps_warm = pssm.tile([128, 4], f32, tag="ps_sel")
            nc.tensor.matmul(ps_warm[0:1, 0:1], lhsT=w_sb[0:1, 0, 0:1],
                             rhs=w_sb[0:1, 0, 0:1], start=True, stop=True)
            nc.tensor.matmul(ps_warm[0:1, 0:1], lhsT=cpack_sb[0:1, 0:1],
                             rhs=cpack_sb[0:1, 0:1], start=True, stop=True)

            def prefix_fix(q):
                qs = slice(q * 32, (q + 1) * 32)
                for dj in range(3):
                    js = slice(dj * 32, (dj + 1) * 32)
                    nc.gpsimd.tensor_copy(out=X3[js, q, 0:4 - dj],
                                          in_=x1q[qs, dj:4])
                    d2 = 2 * dj
                    if d2 < 4:
                        nc.gpsimd.tensor_copy(out=X3e[js, q, 0:4 - d2],
                                              in_=x1q[qs, d2:4])

            def suffix_fix(q):
                qs = slice(q * 32, (q + 1) * 32)
                for dj in range(3):
                    js = slice(dj * 32, (dj + 1) * 32)
                    nc.gpsimd.tensor_copy(
                        out=X3[js, q, LQ + 4 - dj:QF - dj],
                        in_=x1q[qs, LQ + 4:QF])
                    d2 = 2 * dj
                    nc.gpsimd.tensor_copy(
                        out=X3e[js, q, LQ + 4 - d2:QF - d2],
                        in_=x1q[qs, LQ + 4:QF])

            def stage_a(k):
                q = k // 4
                u0 = (k % 4) * LC + 4
                qs = slice(q * 32, (q + 1) * 32)
                xt = xpool.tile([128, NIC, LC], f32, tag="xt")
                nc.sync.dma_start(out=xt, in_=ap["xp"][:, k, :])
                psx = ps1.tile([128, LC], f32, tag="psx")
                # slot-opener: absorbs the PSUM WAR wait so the first real
                # matmul of the chunk only waits on its x-DMA lane
                nc.tensor.matmul(psx[q * 32:q * 32 + 1, 0:1],
                                 lhsT=w_sb[0:1, 0, 0:1], rhs=w_sb[0:1, 0, 0:1],
                                 start=True, stop=True,
                                 tile_position=(0, q * 32))
                for ic in range(NIC):
                    nc.tensor.matmul(psx[qs, :], lhsT=w_sb[:, ic, :],
                                     rhs=xt[:, ic, :],
                                     start=(ic == 0), stop=(ic == NIC - 1),
                                     tile_position=(0, q * 32))
                nc.scalar.activation(out=x1q[qs, u0:u0 + LC], in_=psx[qs, :],
                                     func=AF.Identity, bias=csb["c1b128"][qs, :])
                for dj in range(3):
                    js = slice(dj * 32, (dj + 1) * 32)
                    nc.gpsimd.tensor_copy(
                        out=X3[js, q, u0 - dj:u0 + LC - dj],
                        in_=x1q[qs, u0:u0 + LC])
                    d2 = 2 * dj
                    nc.gpsimd.tensor_copy(
                        out=X3e[js, q, u0 - d2:u0 + LC - d2],
                        in_=x1q[qs, u0:u0 + LC])
                if k % 4 == 0:
                    if q > 0:
                        # right halo of previous quarter, then its suffixes
                        nc.gpsimd.tensor_copy(
                            out=x1q[(q - 1) * 32:q * 32, LQ + 4:QF],
                            in_=x1q[qs, 4:8])
                        suffix_fix(q - 1)
                    prefix_fix(q)
                if k % 4 == 3 and q < 3:
                    nc.gpsimd.tensor_copy(
                        out=x1q[(q + 1) * 32:(q + 2) * 32, 0:4],
                        in_=x1q[qs, LQ:LQ + 4])

            def stage_b(k):
                q, h = k // 4, k // 8
                u0 = (k % 4) * LC + 4
                lc0 = (k % 8) * LC
                hs = slice(h * 64, (h + 1) * 64)
                psa = ps1.tile([128, LC], f32, tag="psa")
                nc.tensor.matmul(psa[h * 64:h * 64 + 1, 0:1],
                                 lhsT=w_sb[0:1, 0, 0:1], rhs=w_sb[0:1, 0, 0:1],
                                 start=True, stop=True,
                                 tile_position=(0, h * 64))
                nc.tensor.matmul(psa[hs, :], lhsT=csb["lhsT_c1"],
                                 rhs=X3[:, q, u0 - 1:u0 - 1 + LC],
                                 start=True, stop=True)
                psb = ps1.tile([128, LC], f32, tag="psb")
                nc.tensor.matmul(psb[h * 64:h * 64 + 1, 0:1],
                                 lhsT=w_sb[0:1, 0, 0:1], rhs=w_sb[0:1, 0, 0:1],
                                 start=True, stop=True,
                                 tile_position=(0, h * 64))
                nc.tensor.matmul(psb[hs, :], lhsT=csb["lhsT_c2"],
                                 rhs=X3e[:, q, u0 - 2:u0 - 2 + LC],
                                 start=True, stop=True)
                nc.scalar.activation(out=a_pre[hs, lc0:lc0 + LC], in_=psa[hs, :],
                                     func=AF.Identity, bias=csb["bvec_c1"][hs, :])
                nc.scalar.activation(out=b_pre[hs, lc0:lc0 + LC], in_=psb[hs, :],
                                     func=AF.Identity, bias=csb["bvec_c2"][hs, :])
                nc.vector.bn_stats(out=st_a[hs, k % 8, :],
                                   in_=a_pre[hs, lc0:lc0 + LC])
                nc.vector.bn_stats(out=st_b[hs, k % 8, :],
                                   in_=b_pre[hs, lc0:lc0 + LC])

            stage_a(0)
            for k in range(1, NCH):
                stage_a(k)
                stage_b(k - 1)
            suffix_fix(3)
            stage_b(NCH - 1)

            # x1 secondary output: [32, 8192] <- quarters
            x1_out_ap = bass.AP(tensor=ap["x1_out"].tensor, offset=0,
                                ap=[[LQ, 4], [L, 32], [1, LQ]])
            nc.sync.dma_start(out=x1_out_ap, in_=x1q[:, 4:4 + LQ])

        # ---- local stats -> (sum, sumsq) per row ----
        def to_sums(st, dst0, dst1, sums, nm):
            mv = small.tile([128, 2], f32, name=f"mv_{nm}")
            nc.vector.bn_aggr(out=mv, in_=st)
            nc.vector.tensor_scalar_mul(out=sums[:, dst0:dst0 + 1],
                                        in0=mv[:, 0:1], scalar1=float(LH))
            m2 = small.tile([128, 1], f32, name=f"m2_{nm}")
            nc.vector.tensor_mul(m2, mv[:, 0:1], mv[:, 0:1])
            e2 = small.tile([128, 1], f32, name=f"e2_{nm}")
            nc.vector.tensor_add(e2, mv[:, 1:2], m2)
            nc.vector.tensor_scalar_mul(out=sums[:, dst1:dst1 + 1],
                                        in0=e2, scalar1=float(LH))

        sums_ab = small.tile([128, 4], f32)
        to_sums(st_a, 0, 1, sums_ab, "a")
        to_sums(st_b, 2, 3, sums_ab, "b")
        ps_sel = pssm.tile([128, 4], f32, tag="ps_sel")
        nc.tensor.matmul(ps_sel, lhsT=csb["lhsT_selB"], rhs=sums_ab,
                         start=True, stop=True)
        exch1 = small.tile([128, 4], f32)
        nc.scalar.activation(out=exch1, in_=ps_sel, func=AF.Copy)
        d1_in = dram.tile([128, 4], f32)
        d1_out = dram.tile([128, 4], f32)
        nc.sync.dma_start(out=d1_in, in_=exch1)
        nc.gpsimd.collective_compute(
            "AllReduce", ALU.add, replica_groups=[list(range(NCORES))],
            ins=[d1_in[:]], outs=[d1_out[:]])
        g_ab_t = small.tile([128, 8], f32)
        g_ab = g_ab_t[:, 0:4]
        nc.sync.dma_start(out=g_ab, in_=d1_out)

        def bn_coeffs(gsum, gsumsq, gb, nm):
            m = small.tile([128, 1], f32, name=f"m_{nm}")
            nc.vector.tensor_scalar_mul(out=m, in0=gsum, scalar1=1.0 / NTOT)
            e2 = small.tile([128, 1], f32, name=f"e2g_{nm}")
            nc.vector.tensor_scalar_mul(out=e2, in0=gsumsq, scalar1=1.0 / NTOT)
            m2 = small.tile([128, 1], f32, name=f"m2g_{nm}")
            nc.vector.tensor_mul(m2, m, m)
            v = small.tile([128, 1], f32, name=f"v_{nm}")
            nc.vector.tensor_sub(v, e2, m2)
            sd = small.tile([128, 1], f32, name=f"sd_{nm}")
            nc.scalar.activation(out=sd, in_=v, func=AF.Sqrt, bias=eps128)
            rs = small.tile([128, 1], f32, name=f"rs_{nm}")
            nc.vector.reciprocal(out=rs, in_=sd)
            scale = small.tile([128, 1], f32, name=f"scale_{nm}")
            nc.vector.tensor_mul(scale, gb[:, 0:1], rs)
            ms = small.tile([128, 1], f32, name=f"ms_{nm}")
            nc.vector.tensor_mul(ms, m, scale)
            bias = small.tile([128, 1], f32, name=f"bias_{nm}")
            nc.vector.tensor_sub(bias, gb[:, 1:2], ms)
            return scale, bias

        scale_a, bias_a = bn_coeffs(g_ab[:, 0:1], g_ab[:, 1:2], csb["gb_a"], "a")
        scale_b, bias_b = bn_coeffs(g_ab[:, 2:3], g_ab[:, 3:4], csb["gb_b"], "b")

        # ================= phase 2 =================
        # relu-apply in place (a_pre/b_pre not needed afterwards)
        nc.scalar.activation(out=a_pre, in_=a_pre, func=AF.Relu,
                             bias=bias_a, scale=scale_a)
        nc.scalar.activation(out=b_pre, in_=b_pre, func=AF.Relu,
                             bias=bias_b, scale=scale_b)
        with ExitStack() as p2:
            ps2 = p2.enter_context(tc.tile_pool(name="ps2", bufs=2, space="PSUM"))
            for k in range(NCH):
                h = k // 8
                hs = slice(h * 64, (h + 1) * 64)
                lc0 = (k % 8) * LC
                psc = ps2.tile([128, LC], f32, tag="psc")
                nc.tensor.matmul(psc[hs, :], lhsT=csb["lhsT_px"][hs, :],
                                 rhs=a_pre[hs, lc0:lc0 + LC],
                                 start=True, stop=False)
                nc.tensor.matmul(psc[hs, :], lhsT=csb["lhsT_px"][hs, :],
                                 rhs=b_pre[hs, lc0:lc0 + LC],
                                 start=False, stop=True)
                nc.scalar.activation(out=c_pre[hs, lc0:lc0 + LC], in_=psc[hs, :],
                                     func=AF.Identity, bias=csb["bvec_px"][hs, :])
                nc.vector.bn_stats(out=st_c[hs, k % 8, :],
                                   in_=c_pre[hs, lc0:lc0 + LC])

        sums_c = small.tile([128, 2], f32)
        to_sums(st_c, 0, 1, sums_c, "c")
        ps_sel2 = pssm.tile([128, 2], f32, tag="ps_sel2")
        nc.tensor.matmul(ps_sel2, lhsT=csb["lhsT_selB"], rhs=sums_c,
                         start=True, stop=True)
        exch2 = small.tile([128, 2], f32)
        nc.scalar.activation(out=exch2, in_=ps_sel2, func=AF.Copy)
        d2_in = dram.tile([128, 2], f32)
        d2_out = dram.tile([128, 2], f32)
        nc.sync.dma_start(out=d2_in, in_=exch2)
        nc.gpsimd.collective_compute(
            "AllReduce", ALU.add, replica_groups=[list(range(NCORES))],
            ins=[d2_in[:]], outs=[d2_out[:]])
        g_c_t = small.tile([128, 8], f32)
        g_c = g_c_t[:, 0:2]
        nc.sync.dma_start(out=g_c, in_=d2_out)
        scale_c, bias_c = bn_coeffs(g_c[:, 0:1], g_c[:, 1:2], csb["gb_c"], "c")

        # ================= phase 3 =================
        p3 = ctx.enter_context(ExitStack())
        ps3 = p3.enter_context(tc.tile_pool(name="ps3", bufs=2, space="PSUM"))
        obuf = p3.enter_context(tc.tile_pool(name="obuf", bufs=3))

        pooled = small.tile([128, 1], f32)
        nc.scalar.activation(out=b_pre, in_=c_pre, func=AF.Relu,
                             bias=bias_c, scale=scale_c, accum_out=pooled)

        ps_w = ps3.tile([4, 1], f32, tag="ps_fc")
        nc.tensor.matmul(ps_w, lhsT=csb["lhsT_fcw"], rhs=pooled,
                         start=True, stop=True)
        width = small.tile([4, 1], f32)
        nc.scalar.activation(out=width, in_=ps_w, func=AF.Identity,
                             bias=csb["fcb_w"])
        ps_c2 = ps3.tile([4, 1], f32, tag="ps_fc")
        nc.tensor.matmul(ps_c2, lhsT=csb["lhsT_fcc"], rhs=pooled,
                         start=True, stop=True)
        center = small.tile([4, 1], f32)
        nc.scalar.activation(out=center, in_=ps_c2, func=AF.Identity,
                             bias=csb["fcb_c"])
        nc.vector.tensor_scalar(out=center, in0=center, scalar1=1.0,
                                scalar2=128.0, op0=ALU.max, op1=ALU.min)

        # t = width * (T - center), T = 0..8
        ii = small.tile([4, K], mybir.dt.int32)
        nc.gpsimd.iota(ii, pattern=[[1, K]], base=0, channel_multiplier=0)
        Tf = small.tile([4, K], f32)
        nc.vector.tensor_copy(out=Tf, in_=ii)
        t = small.tile([4, K], f32)
        nc.vector.tensor_scalar(out=t, in0=Tf, scalar1=center, scalar2=width,
                                op0=ALU.subtract, op1=ALU.mult)

        # Hermite polynomials h0..h6
        hs_t = []
        h0 = small.tile([4, K], f32, name="h0")
        nc.vector.memset(h0, 1.0)
        hs_t.append(h0)
        h1 = small.tile([4, K], f32, name="h1")
        nc.vector.tensor_scalar_mul(out=h1, in0=t, scalar1=2.0)
        hs_t.append(h1)
        for i in range(2, CO - 1):
            tmp = small.tile([4, K], f32, name=f"htmp{i}")
            nc.vector.tensor_mul(tmp, h1, hs_t[-1])
            sc = small.tile([4, K], f32, name=f"hsc{i}")
            nc.vector.tensor_scalar_mul(out=sc, in0=hs_t[-2],
                                        scalar1=2.0 * (i - 1))
            hi = small.tile([4, K], f32, name=f"h{i}")
            nc.vector.tensor_sub(hi, tmp, sc)
            hs_t.append(hi)

        sq = small.tile([4, K], f32)
        nc.scalar.activation(out=sq, in_=t, func=AF.Square)
        e = small.tile([4, K], f32)
        nc.scalar.activation(out=e, in_=sq, func=AF.Exp, scale=-0.5)

        ker = small.tile([4, K * CO], f32)   # layout [b, j*8 + c]
        trash = small.tile([4, K], f32)
        kview = ker.rearrange("b (j c) -> b c j", c=CO)
        for i in range(CO - 1):
            ci = 2.0 ** (i / 2) / math.sqrt(math.pi * math.factorial(i))
            g = small.tile([4, K], f32, name=f"g{i}")
            nc.vector.tensor_mul(g, hs_t[i], e)
            ss = small.tile([4, 1], f32, name=f"ss{i}")
            nc.scalar.activation(out=trash, in_=g, func=AF.Square, accum_out=ss)
            ng = small.tile([4, 1], f32, name=f"ng{i}")
            nc.scalar.activation(out=ng, in_=ss, func=AF.Sqrt)
            den = small.tile([4, 1], f32, name=f"den{i}")
            nc.vector.tensor_scalar(out=den, in0=ng, scalar1=ci,
                                    scalar2=1e-12, op0=ALU.mult, op1=ALU.max)
            rec = small.tile([4, 1], f32, name=f"rec{i}")
            nc.vector.reciprocal(out=rec, in_=den)
            fac = small.tile([4, 1], f32, name=f"fac{i}")
            nc.vector.tensor_scalar_mul(out=fac, in0=rec, scalar1=ci)
            nc.vector.tensor_scalar(out=kview[:, i, :], in0=g, scalar1=fac,
                                    scalar2=None, op0=ALU.mult)
        # f7 = sigmoid(2t) = 1 / (1 + exp(-2t))
        em = small.tile([4, K], f32)
        nc.scalar.activation(out=em, in_=t, func=AF.Exp, scale=-2.0)
        ep1 = small.tile([4, K], f32)
        nc.vector.tensor_scalar_add(out=ep1, in0=em, scalar1=1.0)
        nc.vector.reciprocal(out=kview[:, CO - 1, :], in_=ep1)

        # build the 3 block-diag lhsT [96, 4] with pure matmuls (no partition
        # scatters): transpose ker -> kerT [72,4]; a 0/1 selection matmul
        # replicates group g rows over samples; a block-diag mask multiply
        # zeroes cross-sample entries.
        ps_kt = ps3.tile([72, 4], f32, tag="ps_fc")
        nc.tensor.transpose(ps_kt, ker, csb["ident4"])
        kerT = small.tile([72, 8], f32)
        nc.scalar.activation(out=kerT[:, 0:4], in_=ps_kt, func=AF.Copy)
        ltk = []
        for g3 in range(3):
            ps_lt = ps3.tile([96, 4], f32, tag="ps_fc")
            nc.tensor.matmul(ps_lt, lhsT=csb["sel_dj"][:, 96 * g3:96 * (g3 + 1)],
                             rhs=kerT[:, 0:4], start=True, stop=True)
            lt = small.tile([96, 8], f32, name=f"ltk{g3}")
            nc.vector.tensor_mul(lt[:, 0:4], ps_lt, csb["mask96"])
            ltk.append(lt[:, 0:4])

        if True:
            for k in range(NCH):
                q = k // 4
                u0 = (k % 4) * LC + 4
                l0 = k * LC
                pso = ps3.tile([4, LC], f32, tag="pso")
                for g3 in range(3):
                    nc.tensor.matmul(pso, lhsT=ltk[g3],
                                     rhs=X3[:, q, u0 + 3 * g3 - 4:u0 + 3 * g3 - 4 + LC],
                                     start=(g3 == 0), stop=(g3 == 2))
                ob = obuf.tile([4, LC], f32, tag="ob")
                nc.scalar.activation(out=ob, in_=pso, func=AF.Identity,
                                     bias=csb["c2b4"])
                nc.sync.dma_start(out=ap["out"][:, l0:l0 + LC], in_=ob)

    nc.compile()
    return nc


def _sel_dj():
    # sel[(j,c), g*96 + dj*32 + b*8 + c'] = 1 iff j == 3g+dj and c == c'
    sel = np.zeros((72, 288), np.float32)
    for g in range(3):
        for dj in range(3):
            for b in range(BS):
                for c in range(CO):
                    sel[(3 * g + dj) * CO + c, g * 96 + dj * 32 + b * 8 + c] = 1.0
    return sel


def _mask96():
    m = np.zeros((96, 4), np.float32)
    for dj in range(3):
        for b in range(BS):
            m[dj * 32 + b * 8:dj * 32 + (b + 1) * 8, b] = 1.0
    return m


def _host_prep(inputs):
    """Build per-core input maps from the full problem inputs."""
    f = lambda a: np.asarray(a, dtype=np.float32)
    x = f(inputs["x"])
    W = f(inputs["conv1_w"])[:, :, 0]              # [8, 256]

    wbd = np.zeros((NIC, 128, 32), np.float32)
    for ic in range(NIC):
        blk = W[:, ic * 32:(ic + 1) * 32].T        # [32(isub), 8(o)]
        for b in range(BS):
            wbd[ic, b * 32:(b + 1) * 32, b * 8:(b + 1) * 8] = blk

    c1b128 = np.tile(f(inputs["conv1_b"]), 16).reshape(128, 1)

    def branch(name):
        return f(inputs["w_" + name]), f(inputs["c_" + name])

    def conv_lhsT(wname):
        w_w, w_c = branch(wname)                   # [8(o), 8(c), 3(j)]
        lt = np.zeros((96, 64), np.float32)
        for br, w3 in enumerate((w_w, w_c)):
            for dj in range(3):
                for b in range(BS):
                    lt[dj * 32 + b * 8:dj * 32 + b * 8 + 8,
                       br * 32 + b * 8:br * 32 + b * 8 + 8] = w3[:, :, dj].T
        return lt

    lhsT_c1 = conv_lhsT("c1w")
    lhsT_c2 = conv_lhsT("c2w")

    # rows (h,br,b,o) summed over (h,b); broadcast over cols (h',br',b',o')
    sel64 = np.zeros((64, 64), np.float32)
    for br in range(2):
        for o in range(CO):
            rows = br * 32 + np.arange(BS) * 8 + o
            sel64[np.ix_(rows, rows)] = 1.0
    lhsT_selB = np.tile(sel64, (2, 2))

    px_w, px_c = branch("pxw")
    px64 = np.zeros((64, 64), np.float32)
    for br, w1 in enumerate((px_w, px_c)):
        for b in range(BS):
            px64[br * 32 + b * 8:br * 32 + b * 8 + 8,
                 br * 32 + b * 8:br * 32 + b * 8 + 8] = w1[:, :, 0].T
    lhsT_px = np.concatenate([px64, px64], axis=0)  # [128, 64]

    fcw_w, fcw_c = branch("fcw")                   # [1, 8]
    fcw64 = np.zeros((64, 4), np.float32)
    fcc64 = np.zeros((64, 4), np.float32)
    for b in range(BS):
        fcw64[b * 8:b * 8 + 8, b] = fcw_w[0] / float(L)
        fcc64[32 + b * 8:32 + b * 8 + 8, b] = fcw_c[0] / float(L)
    lhsT_fcw = np.concatenate([fcw64, fcw64], axis=0)
    lhsT_fcc = np.concatenate([fcc64, fcc64], axis=0)

    def vec128(wname):
        w_w, w_c = branch(wname)
        v64 = np.concatenate([np.tile(w_w, BS), np.tile(w_c, BS)])
        return np.tile(v64, 2).reshape(128, 1)

    def gb128(gname, bname):
        return np.concatenate([vec128(gname), vec128(bname)], axis=1)

    cvals = dict(
        c1b128=c1b128, lhsT_c1=lhsT_c1, lhsT_c2=lhsT_c2,
        lhsT_selB=lhsT_selB, lhsT_px=lhsT_px, lhsT_fcw=lhsT_fcw,
        lhsT_fcc=lhsT_fcc,
        bvec_c1=vec128("c1b"), bvec_c2=vec128("c2b"), bvec_px=vec128("pxb"),
        gb_a=gb128("g1", "b1"), gb_b=gb128("g2", "b2"), gb_c=gb128("g3", "b3"),
        fcb_w=np.full((4, 1), f(inputs["w_fcb"])[0], np.float32),
        fcb_c=np.full((4, 1), f(inputs["c_fcb"])[0], np.float32),
        c2b4=np.full((4, 1), f(inputs["conv2_b"])[0], np.float32),
        ident4=np.eye(4, dtype=np.float32),
        sel_dj=_sel_dj(), mask96=_mask96(),
    )
    ncols = sum(v.shape[1] for v in cvals.values())
    cpack = np.zeros((128, ncols), np.float32)
    off = 0
    for name, v in cvals.items():
        p, fdim = v.shape
        cpack[0:p, off:off + fdim] = v
        off += fdim
    shared = dict(wbd=wbd, cpack=cpack)

    in_maps = []
    for core in range(NCORES):
        x4 = x[core * BS:(core + 1) * BS]          # [4, 256, 8192]
        xp = np.ascontiguousarray(
            x4.reshape(BS, NIC, 32, NCH, LC)
              .transpose(0, 2, 3, 1, 4)
              .reshape(128, NCH, NIC * LC))
        in_maps.append(dict(shared, xp=xp))
    return in_maps


def get_nc():
    if "nc" not in _CACHE:
        _CACHE["nc"] = _build_nc()
    return _CACHE["nc"]


def run(inputs, **kw):
    nc = get_nc()
    in_maps = _host_prep(inputs)
    res = run_bass_kernel_spmd(nc, in_maps, core_ids=list(range(NCORES)), **kw)
    out = np.concatenate([res.results[i]["out"][:, None, :]
                          for i in range(NCORES)], axis=0)
    x1 = np.concatenate([res.results[i]["x1_out"].reshape(BS, CO, L)
                         for i in range(NCORES)], axis=0)
    return (out, x1), res


def kernel(**inputs):
    (out, x1), _ = run(inputs)
    return out, x1


# revision 41
# speedup vs baseline: 1.0082x; 1.0082x over previous
"""Trainium2 Bass kernel for nn_HermiteConvolution (dense_cnn).

Data-parallel over batch: 8 NeuronCores x 4 samples each. BatchNorm batch
statistics are made exact with two tiny (<=2KB) cross-core AllReduces.

SBUF layouts (all 128-partition packed; per-partition free bytes are the
scarce resource):
  x1q [128, 2056]  rows (q, b, o): x1 with L split in 4 quarters of 2048,
                   4-col halos on both sides of each quarter.
  X3  [96, 4, 2056] rows (dj, b, c): X3[dj,q,v] = x1q[(q,b,c), v+dj]
  X3e [96, 4, 2056] rows (dj2,b,c): X3e[dj2,q,v] = x1q[(q,b,c), v+2*dj2]
  a_pre/b_pre/c_pre [128, 4096] rows (h, br, b, o): L split in 2 halves.

Per 512-chunk k: q=k//4, h=k//8. Matmul outputs land at partition offsets
q*32 / h*64 (legal PE tile positions), so every compute op is lane-aligned;
only GPSIMD does partition-moving copies (stack builds, halos).

Phases: GEMM+branch convs+bn_stats (overlapped with x-stream) -> AllReduce#1
-> relu+px-conv+bn_stats -> AllReduce#2 -> relu+pool (ACT accum_out), fc,
Hermite kernel [4,9], scatter into 3 block-diag lhsT [96,4], final K=9 conv
as 3 accumulating matmuls per chunk over X3.

Returns (out [32,1,8192], x1 [32,8,8192]) matching the reference tuple.
"""

import math
import numpy as np
from contextlib import ExitStack

import concourse.bass as bass
import concourse.bacc as bacc
import concourse.tile as tile
import concourse.mybir as mybir
from concourse.bass_utils import run_bass_kernel_spmd

f32 = mybir.dt.float32
AF = mybir.ActivationFunctionType
ALU = mybir.AluOpType

NCORES = 8
B = 32
BS = B // NCORES          # 4 samples per core
CIN = 256
CO = 8                    # ORDER / channels
L = 8192
K = 9
LC = 512                  # matmul chunk
NCH = L // LC             # 16
NIC = CIN // 32           # 8 i-chunks of 32
LQ = L // 4               # 2048 per quarter
LH = L // 2               # 4096 per half
QF = LQ + 8               # 2056 quarter free size (4-col halos)
EPS = 1e-5
NTOT = float(B * L)

_CACHE = {}


def _build_nc():
    nc = bacc.Bacc("TRN2", target_bir_lowering=False, debug=False,
                   num_devices=NCORES)

    ap = {}
    def din(name, shape):
        ap[name] = nc.dram_tensor(name, shape, f32, kind="ExternalInput").ap()
    def dout(name, shape):
        ap[name] = nc.dram_tensor(name, shape, f32, kind="ExternalOutput").ap()

    # all small constants ride in one packed tensor -> one DMA -> one
    # semaphore lane (the LDWEIGHTS ISA slot fits a single sync wait)
    cshapes = {
        "c1b128": (128, 1), "lhsT_c1": (96, 64), "lhsT_c2": (96, 64),
        "lhsT_selB": (128, 128), "lhsT_px": (128, 64), "lhsT_fcw": (128, 4),
        "lhsT_fcc": (128, 4), "bvec_c1": (128, 1), "bvec_c2": (128, 1),
        "bvec_px": (128, 1), "gb_a": (128, 2), "gb_b": (128, 2),
        "gb_c": (128, 2), "fcb_w": (4, 1), "fcb_c": (4, 1), "c2b4": (4, 1),
        "ident4": (4, 4), "sel_dj": (72, 288), "mask96": (96, 4),
    }
    coffs = {}
    off = 0
    for name, (p, fdim) in cshapes.items():
        coffs[name] = off
        off += fdim
    CPACK_F = off

    din("xp", [128, NCH, NIC * LC])    # [(b,i32), chunk, (ic, s)] host-prepped
    din("wbd", [NIC, 128, 32])
    din("cpack", [128, CPACK_F])
    dout("out", [BS, L])
    dout("x1_out", [32, L])

    with tile.TileContext(nc) as tc, ExitStack() as ctx:
        const = ctx.enter_context(tc.tile_pool(name="const", bufs=1))
        persist = ctx.enter_context(tc.tile_pool(name="persist", bufs=1))
        small = ctx.enter_context(tc.tile_pool(name="small", bufs=1))
        dram = ctx.enter_context(tc.tile_pool(name="dram", bufs=1, space="DRAM"))
        pssm = ctx.enter_context(tc.tile_pool(name="pssm", bufs=1, space="PSUM"))

        # ---- constant loads (two DMAs total) ----
        w_sb = const.tile([128, NIC, 32], f32)
        nc.sync.dma_start(out=w_sb, in_=ap["wbd"].rearrange("ic p m -> p ic m"))
        cpack_sb = const.tile([128, CPACK_F], f32)
        nc.sync.dma_start(out=cpack_sb, in_=ap["cpack"])
        csb = {name: cpack_sb[0:p, coffs[name]:coffs[name] + fdim]
               for name, (p, fdim) in cshapes.items()}
        eps128 = const.tile([128, 1], f32)
        nc.vector.memset(eps128, EPS)

        # ---- persistent buffers ----
        x1q = persist.tile([128, QF], f32)
        X3 = persist.tile([96, 4, QF], f32)
        X3e = persist.tile([96, 4, QF], f32)
        a_pre = persist.tile([128, LH], f32)
        b_pre = persist.tile([128, LH], f32)
        c_pre = persist.tile([128, LH], f32)
        st_a = persist.tile([128, NCH // 2, 6], f32)
        st_b = persist.tile([128, NCH // 2, 6], f32)
        st_c = persist.tile([128, NCH // 2, 6], f32)

        nc.vector.memset(x1q[0:32, 0:4], 0.0)          # left pad of quarter 0
        nc.vector.memset(x1q[96:128, LQ + 4:QF], 0.0)  # right pad of quarter 3

        # ================= phase 1 =================
        # Stage A(k): x-DMA, conv1 GEMM, x1 chunk, stack copies (+ halo /
        # prefix / suffix fixups at quarter boundaries). Stage B(k): branch
        # convs + stats. B lags A by one chunk: the convs' right taps read
        # the first x1 columns of the NEXT chunk, and Tile preserves
        # program-order semantics, so B(k) must be emitted after A(k+1).
        with ExitStack() as p1:
            xpool = p1.enter_context(tc.tile_pool(name="xpool", bufs=4))
            ps1 = p1.enter_context(tc.tile_pool(name="ps1", bufs=2, space="PSUM"))

            # warm PE's view of the two const DMA lanes so later matmuls
            # (whose LDWEIGHTS slot fits only ONE sync wait) never need a
            # const wait on top of their data wait
            ps_warm = pssm.tile([128, 4], f32, tag="ps_sel")
            nc.tensor.matmul(ps_warm[0:1, 0:1], lhsT=w_sb[0:1, 0, 0:1],
                             rhs=w_sb[0:1, 0, 0:1], start=True, stop=True)
            nc.tensor.matmul(ps_warm[0:1, 0:1], lhsT=cpack_sb[0:1, 0:1],
                             rhs=cpack_sb[0:1, 0:1], start=True, stop=True)

            def prefix_fix(q):
                qs = slice(q * 32, (q + 1) * 32)
                for dj in range(3):
                    js = slice(dj * 32, (dj + 1) * 32)
                    nc.gpsimd.tensor_copy(out=X3[js, q, 0:4 - dj],
                                          in_=x1q[qs, dj:4])
                    d2 = 2 * dj
                    if d2 < 4:
                        nc.gpsimd.tensor_copy(out=X3e[js, q, 0:4 - d2],
                                              in_=x1q[qs, d2:4])

            def suffix_fix(q):
                qs = slice(q * 32, (q + 1) * 32)
                for dj in range(3):
                    js = slice(dj * 32, (dj + 1) * 32)
                    nc.gpsimd.tensor_copy(
                        out=X3[js, q, LQ + 4 - dj:QF - dj],
                        in_=x1q[qs, LQ + 4:QF])
                    d2 = 2 * dj
                    nc.gpsimd.tensor_copy(
                        out=X3e[js, q, LQ + 4 - d2:QF - d2],
                        in_=x1q[qs, LQ + 4:QF])

            def stage_a(k):
                q = k // 4
                u0 = (k % 4) * LC + 4
                qs = slice(q * 32, (q + 1) * 32)
                xt = xpool.tile([128, NIC, LC], f32, tag="xt")
                nc.sync.dma_start(out=xt, in_=ap["xp"][:, k, :])
                psx = ps1.tile([128, LC], f32, tag="psx")
                # slot-opener: absorbs the PSUM WAR wait so the first real
                # matmul of the chunk only waits on its x-DMA lane
                nc.tensor.matmul(psx[q * 32:q * 32 + 1, 0:1],
                                 lhsT=w_sb[0:1, 0, 0:1], rhs=w_sb[0:1, 0, 0:1],
                                 start=True, stop=True,
                                 tile_position=(0, q * 32))
                for ic in range(NIC):
                    nc.tensor.matmul(psx[qs, :], lhsT=w_sb[:, ic, :],
                                     rhs=xt[:, ic, :],
                                     start=(ic == 0), stop=(ic == NIC - 1),
                                     tile_position=(0, q * 32))
                nc.scalar.activation(out=x1q[qs, u0:u0 + LC], in_=psx[qs, :],
                                     func=AF.Identity, bias=csb["c1b128"][qs, :])
                for dj in range(3):
                    js = slice(dj * 32, (dj + 1) * 32)
                    nc.gpsimd.tensor_copy(
                        out=X3[js, q, u0 - dj:u0 + LC - dj],
                        in_=x1q[qs, u0:u0 + LC])
                    d2 = 2 * dj
                    nc.gpsimd.tensor_copy(
                        out=X3e[js, q, u0 - d2:u0 + LC - d2],
                        in_=x1q[qs, u0:u0 + LC])
                if k % 4 == 0:
                    if q > 0:
                        # right halo of previous quarter, then its suffixes
                        nc.gpsimd.tensor_copy(
                            out=x1q[(q - 1) * 32:q * 32, LQ + 4:QF],
                            in_=x1q[qs, 4:8])
                        suffix_fix(q - 1)
                    prefix_fix(q)
                if k % 4 == 3 and q < 3:
                    nc.gpsimd.tensor_copy(
                        out=x1q[(q + 1) * 32:(q + 2) * 32, 0:4],
                        in_=x1q[qs, LQ:LQ + 4])

            def stage_b(k):
                q, h = k // 4, k // 8
                u0 = (k % 4) * LC + 4
                lc0 = (k % 8) * LC
                hs = slice(h * 64, (h + 1) * 64)
                psa = ps1.tile([128, LC], f32, tag="psa")
                nc.tensor.matmul(psa[h * 64:h * 64 + 1, 0:1],
                                 lhsT=w_sb[0:1, 0, 0:1], rhs=w_sb[0:1, 0, 0:1],
                                 start=True, stop=True,
                                 tile_position=(0, h * 64))
                nc.tensor.matmul(psa[hs, :], lhsT=csb["lhsT_c1"],
                                 rhs=X3[:, q, u0 - 1:u0 - 1 + LC],
                                 start=True, stop=True)
                psb = ps1.tile([128, LC], f32, tag="psb")
                nc.tensor.matmul(psb[h * 64:h * 64 + 1, 0:1],
                                 lhsT=w_sb[0:1, 0, 0:1], rhs=w_sb[0:1, 0, 0:1],
                                 start=True, stop=True,
                                 tile_position=(0, h * 64))
                nc.tensor.matmul(psb[hs, :], lhsT=csb["lhsT_c2"],
                                 rhs=X3e[:, q, u0 - 2:u0 - 2 + LC],
                                 start=True, stop=True)
                nc.scalar.activation(out=a_pre[hs, lc0:lc0 + LC], in_=psa[hs, :],
                                     func=AF.Identity, bias=csb["bvec_c1"][hs, :])
                nc.scalar.activation(out=b_pre[hs, lc0:lc0 + LC], in_=psb[hs, :],
                                     func=AF.Identity, bias=csb["bvec_c2"][hs, :])
                nc.vector.bn_stats(out=st_a[hs, k % 8, :],
                                   in_=a_pre[hs, lc0:lc0 + LC])
                nc.vector.bn_stats(out=st_b[hs, k % 8, :],
                                   in_=b_pre[hs, lc0:lc0 + LC])

            stage_a(0)
            for k in range(1, NCH):
                stage_a(k)
                stage_b(k - 1)
            suffix_fix(3)
            stage_b(NCH - 1)

            # x1 secondary output: [32, 8192] <- quarters
            x1_out_ap = bass.AP(tensor=ap["x1_out"].tensor, offset=0,
                                ap=[[LQ, 4], [L, 32], [1, LQ]])
            nc.sync.dma_start(out=x1_out_ap, in_=x1q[:, 4:4 + LQ])

        # ---- local stats -> (sum, sumsq) per row ----
        def to_sums(st, dst0, dst1, sums, nm):
            mv = small.tile([128, 2], f32, name=f"mv_{nm}")
            nc.vector.bn_aggr(out=mv, in_=st)
            nc.vector.tensor_scalar_mul(out=sums[:, dst0:dst0 + 1],
                                        in0=mv[:, 0:1], scalar1=float(LH))
            m2 = small.tile([128, 1], f32, name=f"m2_{nm}")
            nc.vector.tensor_mul(m2, mv[:, 0:1], mv[:, 0:1])
            e2 = small.tile([128, 1], f32, name=f"e2_{nm}")
            nc.vector.tensor_add(e2, mv[:, 1:2], m2)
            nc.vector.tensor_scalar_mul(out=sums[:, dst1:dst1 + 1],
                                        in0=e2, scalar1=float(LH))

        sums_ab = small.tile([128, 4], f32)
        to_sums(st_a, 0, 1, sums_ab, "a")
        to_sums(st_b, 2, 3, sums_ab, "b")
        ps_sel = pssm.tile([128, 4], f32, tag="ps_sel")
        nc.tensor.matmul(ps_sel, lhsT=csb["lhsT_selB"], rhs=sums_ab,
                         start=True, stop=True)
        exch1 = small.tile([128, 4], f32)
        nc.scalar.activation(out=exch1, in_=ps_sel, func=AF.Copy)
        d1_in = dram.tile([128, 4], f32)
        d1_out = dram.tile([128, 4], f32)
        nc.sync.dma_start(out=d1_in, in_=exch1)
        nc.gpsimd.collective_compute(
            "AllReduce", ALU.add, replica_groups=[list(range(NCORES))],
            ins=[d1_in[:]], outs=[d1_out[:]])
        g_ab_t = small.tile([128, 8], f32)
        g_ab = g_ab_t[:, 0:4]
        nc.sync.dma_start(out=g_ab, in_=d1_out)

        def bn_coeffs(gsum, gsumsq, gb, nm):
            m = small.tile([128, 1], f32, name=f"m_{nm}")
            nc.vector.tensor_scalar_mul(out=m, in0=gsum, scalar1=1.0 / NTOT)
            e2 = small.tile([128, 1], f32, name=f"e2g_{nm}")
            nc.vector.tensor_scalar_mul(out=e2, in0=gsumsq, scalar1=1.0 / NTOT)
            m2 = small.tile([128, 1], f32, name=f"m2g_{nm}")
            nc.vector.tensor_mul(m2, m, m)
            v = small.tile([128, 1], f32, name=f"v_{nm}")
            nc.vector.tensor_sub(v, e2, m2)
            sd = small.tile([128, 1], f32, name=f"sd_{nm}")
            nc.scalar.activation(out=sd, in_=v, func=AF.Sqrt, bias=eps128)
            rs = small.tile([128, 1], f32, name=f"rs_{nm}")
            nc.vector.reciprocal(out=rs, in_=sd)
            scale = small.tile([128, 1], f32, name=f"scale_{nm}")
            nc.vector.tensor_mul(scale, gb[:, 0:1], rs)
            ms = small.tile([128, 1], f32, name=f"ms_{nm}")
            nc.vector.tensor_mul(ms, m, scale)
            bias = small.tile([128, 1], f32, name=f"bias_{nm}")
            nc.vector.tensor_sub(bias, gb[:, 1:2], ms)
            return scale, bias

        scale_a, bias_a = bn_coeffs(g_ab[:, 0:1], g_ab[:, 1:2], csb["gb_a"], "a")
        scale_b, bias_b = bn_coeffs(g_ab[:, 2:3], g_ab[:, 3:4], csb["gb_b"], "b")

        # ================= phase 2 =================
        # relu-apply in place (a_pre/b_pre not needed afterwards); a on ACT,
        # b on DVE so the two passes run on different engines
        nc.scalar.activation(out=a_pre, in_=a_pre, func=AF.Relu,
                             bias=bias_a, scale=scale_a)
        nc.vector.tensor_scalar(out=b_pre, in0=b_pre, scalar1=scale_b,
                                scalar2=bias_b, op0=ALU.mult, op1=ALU.add)
        nc.vector.tensor_scalar_max(out=b_pre, in0=b_pre, scalar1=0.0)
        with ExitStack() as p2:
            ps2 = p2.enter_context(tc.tile_pool(name="ps2", bufs=2, space="PSUM"))
            for k in range(NCH):
                h = k // 8
                hs = slice(h * 64, (h + 1) * 64)
                lc0 = (k % 8) * LC
                psc = ps2.tile([128, LC], f32, tag="psc")
                nc.tensor.matmul(psc[hs, :], lhsT=csb["lhsT_px"][hs, :],
                                 rhs=a_pre[hs, lc0:lc0 + LC],
                                 start=True, stop=False)
                nc.tensor.matmul(psc[hs, :], lhsT=csb["lhsT_px"][hs, :],
                                 rhs=b_pre[hs, lc0:lc0 + LC],
                                 start=False, stop=True)
                nc.scalar.activation(out=c_pre[hs, lc0:lc0 + LC], in_=psc[hs, :],
                                     func=AF.Identity, bias=csb["bvec_px"][hs, :])
                nc.vector.bn_stats(out=st_c[hs, k % 8, :],
                                   in_=c_pre[hs, lc0:lc0 + LC])

        sums_c = small.tile([128, 2], f32)
        to_sums(st_c, 0, 1, sums_c, "c")
        ps_sel2 = pssm.tile([128, 2], f32, tag="ps_sel2")
        nc.tensor.matmul(ps_sel2, lhsT=csb["lhsT_selB"], rhs=sums_c,
                         start=True, stop=True)
        exch2 = small.tile([128, 2], f32)
        nc.scalar.activation(out=exch2, in_=ps_sel2, func=AF.Copy)
        d2_in = dram.tile([128, 2], f32)
        d2_out = dram.tile([128, 2], f32)
        nc.sync.dma_start(out=d2_in, in_=exch2)
        nc.gpsimd.collective_compute(
            "AllReduce", ALU.add, replica_groups=[list(range(NCORES))],
            ins=[d2_in[:]], outs=[d2_out[:]])
        g_c_t = small.tile([128, 8], f32)
        g_c = g_c_t[:, 0:2]
        nc.sync.dma_start(out=g_c, in_=d2_out)
        scale_c, bias_c = bn_coeffs(g_c[:, 0:1], g_c[:, 1:2], csb["gb_c"], "c")

        # ================= phase 3 =================
        p3 = ctx.enter_context(ExitStack())
        ps3 = p3.enter_context(tc.tile_pool(name="ps3", bufs=2, space="PSUM"))
        obuf = p3.enter_context(tc.tile_pool(name="obuf", bufs=3))

        pooled = small.tile([128, 1], f32)
        nc.scalar.activation(out=b_pre, in_=c_pre, func=AF.Relu,
                             bias=bias_c, scale=scale_c, accum_out=pooled)

        ps_w = ps3.tile([4, 1], f32, tag="ps_fc")
        nc.tensor.matmul(ps_w, lhsT=csb["lhsT_fcw"], rhs=pooled,
                         start=True, stop=True)
        width = small.tile([4, 1], f32)
        nc.scalar.activation(out=width, in_=ps_w, func=AF.Identity,
                             bias=csb["fcb_w"])
        ps_c2 = ps3.tile([4, 1], f32, tag="ps_fc")
        nc.tensor.matmul(ps_c2, lhsT=csb["lhsT_fcc"], rhs=pooled,
                         start=True, stop=True)
        center = small.tile([4, 1], f32)
        nc.scalar.activation(out=center, in_=ps_c2, func=AF.Identity,
                             bias=csb["fcb_c"])
        nc.vector.tensor_scalar(out=center, in0=center, scalar1=1.0,
                                scalar2=128.0, op0=ALU.max, op1=ALU.min)

        # t = width * (T - center), T = 0..8
        ii = small.tile([4, K], mybir.dt.int32)
        nc.gpsimd.iota(ii, pattern=[[1, K]], base=0, channel_multiplier=0)
        Tf = small.tile([4, K], f32)
        nc.vector.tensor_copy(out=Tf, in_=ii)
        t = small.tile([4, K], f32)
        nc.vector.tensor_scalar(out=t, in0=Tf, scalar1=center, scalar2=width,
                                op0=ALU.subtract, op1=ALU.mult)

        # Hermite polynomials h0..h6
        hs_t = []
        h0 = small.tile([4, K], f32, name="h0")
        nc.vector.memset(h0, 1.0)
        hs_t.append(h0)
        h1 = small.tile([4, K], f32, name="h1")
        nc.vector.tensor_scalar_mul(out=h1, in0=t, scalar1=2.0)
        hs_t.append(h1)
        for i in range(2, CO - 1):
            tmp = small.tile([4, K], f32, name=f"htmp{i}")
            nc.vector.tensor_mul(tmp, h1, hs_t[-1])
            sc = small.tile([4, K], f32, name=f"hsc{i}")
            nc.vector.tensor_scalar_mul(out=sc, in0=hs_t[-2],
                                        scalar1=2.0 * (i - 1))
            hi = small.tile([4, K], f32, name=f"h{i}")
            nc.vector.tensor_sub(hi, tmp, sc)
            hs_t.append(hi)

        sq = small.tile([4, K], f32)
        nc.scalar.activation(out=sq, in_=t, func=AF.Square)
        e = small.tile([4, K], f32)
        nc.scalar.activation(out=e, in_=sq, func=AF.Exp, scale=-0.5)

        ker = small.tile([4, K * CO], f32)   # layout [b, j*8 + c]
        trash = small.tile([4, K], f32)
        kview = ker.rearrange("b (j c) -> b c j", c=CO)
        for i in range(CO - 1):
            ci = 2.0 ** (i / 2) / math.sqrt(math.pi * math.factorial(i))
            g = small.tile([4, K], f32, name=f"g{i}")
            nc.vector.tensor_mul(g, hs_t[i], e)
            ss = small.tile([4, 1], f32, name=f"ss{i}")
            nc.scalar.activation(out=trash, in_=g, func=AF.Square, accum_out=ss)
            ng = small.tile([4, 1], f32, name=f"ng{i}")
            nc.scalar.activation(out=ng, in_=ss, func=AF.Sqrt)
            den = small.tile([4, 1], f32, name=f"den{i}")
            nc.vector.tensor_scalar(out=den, in0=ng, scalar1=ci,
                                    scalar2=1e-12, op0=ALU.mult, op1=ALU.max)
            rec = small.tile([4, 1], f32, name=f"rec{i}")
            nc.vector.reciprocal(out=rec, in_=den)
            fac = small.tile([4, 1], f32, name=f"fac{i}")
            nc.vector.tensor_scalar_mul(out=fac, in0=rec, scalar1=ci)
            nc.vector.tensor_scalar(out=kview[:, i, :], in0=g, scalar1=fac,
                                    scalar2=None, op0=ALU.mult)
        # f7 = sigmoid(2t) = 1 / (1 + exp(-2t))
        em = small.tile([4, K], f32)
        nc.scalar.activation(out=em, in_=t, func=AF.Exp, scale=-2.0)
        ep1 = small.tile([4, K], f32)
        nc.vector.tensor_scalar_add(out=ep1, in0=em, scalar1=1.0)
        nc.vector.reciprocal(out=kview[:, CO - 1, :], in_=ep1)

        # build the 3 block-diag lhsT [96, 4] with pure matmuls (no partition
        # scatters): transpose ker -> kerT [72,4]; a 0/1 selection matmul
        # replicates group g rows over samples; a block-diag mask multiply
        # zeroes cross-sample entries.
        ps_kt = ps3.tile([72, 4], f32, tag="ps_fc")
        nc.tensor.transpose(ps_kt, ker, csb["ident4"])
        kerT = small.tile([72, 8], f32)
        nc.scalar.activation(out=kerT[:, 0:4], in_=ps_kt, func=AF.Copy)
        ltk = []
        for g3 in range(3):
            ps_lt = ps3.tile([96, 4], f32, tag="ps_fc")
            nc.tensor.matmul(ps_lt, lhsT=csb["sel_dj"][:, 96 * g3:96 * (g3 + 1)],
                             rhs=kerT[:, 0:4], start=True, stop=True)
            lt = small.tile([96, 8], f32, name=f"ltk{g3}")
            nc.vector.tensor_mul(lt[:, 0:4], ps_lt, csb["mask96"])
            ltk.append(lt[:, 0:4])

        if True:
            for k in range(NCH):
                q = k // 4
                u0 = (k % 4) * LC + 4
                l0 = k * LC
                pso = ps3.tile([4, LC], f32, tag="pso")
                for g3 in range(3):
                    nc.tensor.matmul(pso, lhsT=ltk[g3],
                                     rhs=X3[:, q, u0 + 3 * g3 - 4:u0 + 3 * g3 - 4 + LC],
                                     start=(g3 == 0), stop=(g3 == 2))
                ob = obuf.tile([4, LC], f32, tag="ob")
                nc.scalar.activation(out=ob, in_=pso, func=AF.Identity,
                                     bias=csb["c2b4"])
                nc.sync.dma_start(out=ap["out"][:, l0:l0 + LC], in_=ob)

    nc.compile()
    return nc


def _sel_dj():
    # sel[(j,c), g*96 + dj*32 + b*8 + c'] = 1 iff j == 3g+dj and c == c'
    sel = np.zeros((72, 288), np.float32)
    for g in range(3):
        for dj in range(3):
            for b in range(BS):
                for c in range(CO):
                    sel[(3 * g + dj) * CO + c, g * 96 + dj * 32 + b * 8 + c] = 1.0
    return sel


def _mask96():
    m = np.zeros((96, 4), np.float32)
    for dj in range(3):
        for b in range(BS):
            m[dj * 32 + b * 8:dj * 32 + (b + 1) * 8, b] = 1.0
    return m


def _host_prep(inputs):
    """Build per-core input maps from the full problem inputs."""
    f = lambda a: np.asarray(a, dtype=np.float32)
    x = f(inputs["x"])
    W = f(inputs["conv1_w"])[:, :, 0]              # [8, 256]

    wbd = np.zeros((NIC, 128, 32), np.float32)
    for ic in range(NIC):
        blk = W[:, ic * 32:(ic + 1) * 32].T        # [32(isub), 8(o)]
        for b in range(BS):
            wbd[ic, b * 32:(b + 1) * 32, b * 8:(b + 1) * 8] = blk

    c1b128 = np.tile(f(inputs["conv1_b"]), 16).reshape(128, 1)

    def branch(name):
        return f(inputs["w_" + name]), f(inputs["c_" + name])

    def conv_lhsT(wname):
        w_w, w_c = branch(wname)                   # [8(o), 8(c), 3(j)]
        lt = np.zeros((96, 64), np.float32)
        for br, w3 in enumerate((w_w, w_c)):
            for dj in range(3):
                for b in range(BS):
                    lt[dj * 32 + b * 8:dj * 32 + b * 8 + 8,
                       br * 32 + b * 8:br * 32 + b * 8 + 8] = w3[:, :, dj].T
        return lt

    lhsT_c1 = conv_lhsT("c1w")
    lhsT_c2 = conv_lhsT("c2w")

    # rows (h,br,b,o) summed over (h,b); broadcast over cols (h',br',b',o')
    sel64 = np.zeros((64, 64), np.float32)
    for br in range(2):
        for o in range(CO):
            rows = br * 32 + np.arange(BS) * 8 + o
            sel64[np.ix_(rows, rows)] = 1.0
    lhsT_selB = np.tile(sel64, (2, 2))

    px_w, px_c = branch("pxw")
    px64 = np.zeros((64, 64), np.float32)
    for br, w1 in enumerate((px_w, px_c)):
        for b in range(BS):
            px64[br * 32 + b * 8:br * 32 + b * 8 + 8,
                 br * 32 + b * 8:br * 32 + b * 8 + 8] = w1[:, :, 0].T
    lhsT_px = np.concatenate([px64, px64], axis=0)  # [128, 64]

    fcw_w, fcw_c = branch("fcw")                   # [1, 8]
    fcw64 = np.zeros((64, 4), np.float32)
    fcc64 = np.zeros((64, 4), np.float32)
    for b in range(BS):
        fcw64[b * 8:b * 8 + 8, b] = fcw_w[0] / float(L)
        fcc64[32 + b * 8:32 + b * 8 + 8, b] = fcw_c[0] / float(L)
    lhsT_fcw = np.concatenate([fcw64, fcw64], axis=0)
    lhsT_fcc = np.concatenate([fcc64, fcc64], axis=0)

    def vec128(wname):
        w_w, w_c = branch(wname)
        v64 = np.concatenate([np.tile(w_w, BS), np.tile(w_c, BS)])
        return np.tile(v64, 2).reshape(128, 1)

    def gb128(gname, bname):
        return np.concatenate([vec128(gname), vec128(bname)], axis=1)

    cvals = dict(
        c1b128=c1b128, lhsT_c1=lhsT_c1, lhsT_c2=lhsT_c2,
        lhsT_selB=lhsT_selB, lhsT_px=lhsT_px, lhsT_fcw=lhsT_fcw,
        lhsT_fcc=lhsT_fcc,
        bvec_c1=vec128("c1b"), bvec_c2=vec128("c2b"), bvec_px=vec128("pxb"),
        gb_a=gb128("g1", "b1"), gb_b=gb128("g2", "b2"), gb_c=gb128("g3", "b3"),
        fcb_w=np.full((4, 1), f(inputs["w_fcb"])[0], np.float32),
        fcb_c=np.full((4, 1), f(inputs["c_fcb"])[0], np.float32),
        c2b4=np.full((4, 1), f(inputs["conv2_b"])[0], np.float32),
        ident4=np.eye(4, dtype=np.float32),
        sel_dj=_sel_dj(), mask96=_mask96(),
    )
    ncols = sum(v.shape[1] for v in cvals.values())
    cpack = np.zeros((128, ncols), np.float32)
    off = 0
    for name, v in cvals.items():
        p, fdim = v.shape
        cpack[0:p, off:off + fdim] = v
        off += fdim
    shared = dict(wbd=wbd, cpack=cpack)

    in_maps = []
    for core in range(NCORES):
        x4 = x[core * BS:(core + 1) * BS]          # [4, 256, 8192]
        xp = np.ascontiguousarray(
            x4.reshape(BS, NIC, 32, NCH, LC)
              .transpose(0, 2, 3, 1, 4)
              .reshape(128, NCH, NIC * LC))
        in_maps.append(dict(shared, xp=xp))
    return in_maps


def get_nc():
    if "nc" not in _CACHE:
        _CACHE["nc"] = _build_nc()
    return _CACHE["nc"]


def run(inputs, **kw):
    nc = get_nc()
    in_maps = _host_prep(inputs)
    res = run_bass_kernel_spmd(nc, in_maps, core_ids=list(range(NCORES)), **kw)
    out = np.concatenate([res.results[i]["out"][:, None, :]
                          for i in range(NCORES)], axis=0)
    x1 = np.concatenate([res.results[i]["x1_out"].reshape(BS, CO, L)
                         for i in range(NCORES)], axis=0)
    return (out, x1), res


def kernel(**inputs):
    (out, x1), _ = run(inputs)
    return out, x1
